# revision 5
# baseline (speedup 1.0000x reference)
"""NoisyDense forward for Trainium2, 8-core tensor-parallel.

out = relu(x @ (w_mu + w_sigma * outer(eps_in, eps_out)) + b_mu + b_sigma*eps_out)

Sharding: 2-way over batch x 4-way over units (8 cores).
Per core: x_shard [2048, 4096] (batch rows), w shards [4096, 1024] (unit cols).
On-chip per core:
  - materialize noisy W shard once in SBUF (2 DVE ops/elem, 128KB/partition)
  - stream x in 128-row panels, PE-transpose 128x128 tiles (fp32 has no DMA
    transpose; PE transpose-mode matmul), fp32r matmuls (1 cyc/row @ N=512)
  - bias add + relu on DVE during PSUM eviction

fp32r note: the BIR verifier requires every producer of an fp32r-matmul
operand to emit dtype float32r itself (engines round on write), so the
x / w_mu DRAM tensors and all tiles on the matmul path are float32r
end-to-end. numpy view is float32 either way.
"""

import numpy as np

BATCH = 4096
IN_DIM = 4096
UNITS = 4096
MSHARDS = 2
NSHARDS = 4
MS = BATCH // MSHARDS      # 2048 rows of x per core
NS = UNITS // NSHARDS      # 1024 units per core
P = 128
KO = IN_DIM // P           # 32 k-tiles
MP = MS // P               # 16 m-panels per core
NFREE = 512                # matmul moving free dim (one PSUM bank of fp32)
NT = NS // NFREE           # 2 n-tiles per core

_NC_CACHE = {}


def _build(mm_dtype_name="float32r"):
    from concourse import bacc
    import concourse.mybir as mybir
    import concourse.tile as tile
    from concourse.masks import make_identity

    f32 = mybir.dt.float32
    mdt = getattr(mybir.dt, mm_dtype_name)

    nc = bacc.Bacc(None, target_bir_lowering=False)

    x_d = nc.dram_tensor("x_s", [MS, IN_DIM], mdt, kind="ExternalInput")
    wmu_d = nc.dram_tensor("wmu_s", [IN_DIM, NS], mdt, kind="ExternalInput")
    wsig_d = nc.dram_tensor("wsig_s", [IN_DIM, NS], f32, kind="ExternalInput")
    bmu_d = nc.dram_tensor("bmu_s", [NS], f32, kind="ExternalInput")
    bsig_d = nc.dram_tensor("bsig_s", [NS], f32, kind="ExternalInput")
    eout_d = nc.dram_tensor("eout_s", [NS], f32, kind="ExternalInput")
    ein_d = nc.dram_tensor("eps_in", [IN_DIM], f32, kind="ExternalInput")
    out_d = nc.dram_tensor("out_s", [MS, NS], f32, kind="ExternalOutput")

    mult = mybir.AluOpType.mult

    with tile.TileContext(nc) as tc:
        with (
            tc.tile_pool(name="const", bufs=1) as const,
            tc.tile_pool(name="wpool", bufs=1) as wpool,
            tc.tile_pool(name="wsig", bufs=1) as wsigp,
            tc.tile_pool(name="xnat", bufs=1) as xnat,
            tc.tile_pool(name="xt", bufs=2) as xtp,
            tc.tile_pool(name="outp", bufs=2) as outp,
            tc.tile_pool(name="ps", bufs=3, space="PSUM") as psp,
            tc.tile_pool(name="pt", bufs=4, space="PSUM") as ptp,
        ):
            # ---- constants ----
            ident_f = const.tile([P, P], f32, tag="identf")
            make_identity(nc, ident_f)
            if mdt != f32:
                ident = const.tile([P, P], mdt, tag="ident")
                nc.vector.tensor_copy(out=ident[:], in_=ident_f[:])
            else:
                ident = ident_f

            eps_in_sb = const.tile([P, KO], f32, tag="epsin")
            with nc.allow_non_contiguous_dma(reason="one-time 16KB strided load"):
                nc.sync.dma_start(
                    eps_in_sb[:], ein_d[:].bitcast(f32).rearrange("(ko ki) -> ki ko", ki=P)
                )

            eout_row = const.tile([1, NS], f32, tag="eoutr")
            nc.sync.dma_start(eout_row[:], eout_d[None, :])
            bmu_row = const.tile([1, NS], f32, tag="bmur")
            nc.sync.dma_start(bmu_row[:], bmu_d[None, :])
            bsig_row = const.tile([1, NS], f32, tag="bsigr")
            nc.sync.dma_start(bsig_row[:], bsig_d[None, :])

            eout_b = const.tile([P, NS], f32, tag="eoutb")
            nc.gpsimd.partition_broadcast(eout_b[:], eout_row[:])

            # b = b_mu + b_sigma * eps_out  (computed on partition 0, then bcast)
            nc.vector.tensor_mul(bsig_row[:], bsig_row[:], eout_row[:])
            nc.vector.tensor_add(bmu_row[:], bmu_row[:], bsig_row[:])
            b_b = const.tile([P, NS], f32, tag="bb")
            nc.gpsimd.partition_broadcast(b_b[:], bmu_row[:])

            # ---- materialize noisy W shard in SBUF: w = w_mu + w_sigma*outer ----
            w_tiles = []
            for ko in range(KO):
                wt = wpool.tile([P, NS], mdt, tag=f"w{ko}")
                nc.sync.dma_start(wt[:], wmu_d[ko * P : (ko + 1) * P, :])
                ws = wsigp.tile([P, NS], f32, tag="ws")
                nc.sync.dma_start(ws[:], wsig_d[ko * P : (ko + 1) * P, :])
                # ws = (eps_out_bcast * eps_in[:,ko]) * ws
                nc.vector.scalar_tensor_tensor(
                    out=ws[:],
                    in0=eout_b[:],
                    scalar=eps_in_sb[:, ko : ko + 1],
                    in1=ws[:],
                    op0=mult,
                    op1=mult,
                )
                nc.vector.tensor_add(wt[:], wt[:], ws[:])
                w_tiles.append(wt)

            # ---- panels: transpose x tiles on PE, then fp32r matmuls ----
            def make_transpose_ops(pm):
                xa = xnat.tile([P, IN_DIM // 2], mdt, tag="xa")
                nc.sync.dma_start(
                    xa[:], x_d[pm * P : (pm + 1) * P, 0 : IN_DIM // 2]
                )
                xb = xnat.tile([P, IN_DIM // 2], mdt, tag="xb")
                nc.sync.dma_start(
                    xb[:], x_d[pm * P : (pm + 1) * P, IN_DIM // 2 : IN_DIM]
                )
                xts = [None] * KO
                ops = []

                def mk(ko):
                    def op():
                        half = xa if ko < KO // 2 else xb
                        j = ko % (KO // 2)
                        src = half[:, j * P : (j + 1) * P]
                        pt = ptp.tile([P, P], mdt, tag="pt")
                        nc.tensor.transpose(pt[:], src, ident[:])
                        t = xtp.tile([P, P], mdt, tag=f"xt{ko}")
                        if ko % 2 == 0:
                            nc.vector.tensor_copy(out=t[:], in_=pt[:])
                        else:
                            nc.scalar.copy(out=t[:], in_=pt[:])
                        xts[ko] = t

                    return op

                for ko in range(KO):
                    ops.append(mk(ko))
                return ops, xts

            prev_xts = None
            for mi in range(MP + 1):
                if mi < MP:
                    t_ops, cur_xts = make_transpose_ops(mi)
                else:
                    t_ops, cur_xts = [], None

                if prev_xts is None:
                    for op in t_ops:
                        op()
                else:
                    pm = mi - 1
                    ti = 0
                    for nt in range(NT):
                        ps = psp.tile([P, NFREE], f32, tag="ps")
                        for ko in range(KO):
                            nc.tensor.matmul(
                                ps[:],
                                prev_xts[ko][:],
                                w_tiles[ko][:, nt * NFREE : (nt + 1) * NFREE],
                                start=(ko == 0),
                                stop=(ko == KO - 1),
                            )
                            if ko % 2 == 1 and ti < len(t_ops):
                                t_ops[ti]()
                                ti += 1
                        ot = outp.tile([P, NFREE], f32, tag="ot")
                        nc.vector.tensor_add(
                            ot[:], ps[:], b_b[:, nt * NFREE : (nt + 1) * NFREE]
                        )
                        nc.vector.tensor_scalar_max(ot[:], ot[:], 0.0)
                        nc.sync.dma_start(
                            out_d[pm * P : (pm + 1) * P, nt * NFREE : (nt + 1) * NFREE],
                            ot[:],
                        )
                    while ti < len(t_ops):
                        t_ops[ti]()
                        ti += 1
                prev_xts = cur_xts

    nc.compile()
    return nc


def get_nc(mm_dtype_name="float32r"):
    if mm_dtype_name not in _NC_CACHE:
        _NC_CACHE[mm_dtype_name] = _build(mm_dtype_name)
    return _NC_CACHE[mm_dtype_name]


def shard_inputs(x, w_mu, w_sigma, b_mu, b_sigma, eps_in, eps_out):
    x = np.asarray(x, dtype=np.float32)
    w_mu = np.asarray(w_mu, dtype=np.float32)
    w_sigma = np.asarray(w_sigma, dtype=np.float32)
    b_mu = np.asarray(b_mu, dtype=np.float32)
    b_sigma = np.asarray(b_sigma, dtype=np.float32)
    eps_in = np.asarray(eps_in, dtype=np.float32)
    eps_out = np.asarray(eps_out, dtype=np.float32)

    in_maps = []
    for c in range(MSHARDS * NSHARDS):
        mr, ncol = divmod(c, NSHARDS)
        msl = slice(mr * MS, (mr + 1) * MS)
        nsl = slice(ncol * NS, (ncol + 1) * NS)
        in_maps.append(
            {
                "x_s": np.ascontiguousarray(x[msl, :]),
                "wmu_s": np.ascontiguousarray(w_mu[:, nsl]),
                "wsig_s": np.ascontiguousarray(w_sigma[:, nsl]),
                "bmu_s": np.ascontiguousarray(b_mu[nsl]),
                "bsig_s": np.ascontiguousarray(b_sigma[nsl]),
                "eout_s": np.ascontiguousarray(eps_out[nsl]),
                "eps_in": eps_in,
            }
        )
    return in_maps


def unshard_output(results):
    out = np.empty((BATCH, UNITS), dtype=np.float32)
    for c, rmap in enumerate(results):
        mr, ncol = divmod(c, NSHARDS)
        out[mr * MS : (mr + 1) * MS, ncol * NS : (ncol + 1) * NS] = rmap["out_s"]
    return out


def kernel(x, w_mu, w_sigma, b_mu, b_sigma, eps_in, eps_out):
    from concourse.bass_utils import run_bass_kernel_spmd

    nc = get_nc()
    in_maps = shard_inputs(x, w_mu, w_sigma, b_mu, b_sigma, eps_in, eps_out)
    res = run_bass_kernel_spmd(nc, in_maps, core_ids=list(range(8)))
    return unshard_output(res.results)


# revision 7
# speedup vs baseline: 1.2492x; 1.2492x over previous
"""NoisyDense forward for Trainium2, 8-core tensor-parallel.

out = relu(x @ (w_mu + w_sigma * outer(eps_in, eps_out)) + b_mu + b_sigma*eps_out)

Sharding: 2-way over batch x 4-way over units (8 cores).
Per core: x_shard [2048, 4096] (batch rows), w shards [4096, 1024] (unit cols).
On-chip per core:
  - materialize noisy W shard once in SBUF (2 DVE ops/elem, 128KB/partition)
  - stream x in 128-row panels, PE-transpose 128x128 tiles (fp32 has no DMA
    transpose; PE transpose-mode matmul), fp32r matmuls (1 cyc/row @ N=512)
  - bias add + relu on DVE during PSUM eviction

fp32r note: the BIR verifier requires every producer of an fp32r-matmul
operand to emit dtype float32r itself (engines round on write), so the
x / w_mu DRAM tensors and all tiles on the matmul path are float32r
end-to-end. numpy view is float32 either way.
"""

import numpy as np

BATCH = 4096
IN_DIM = 4096
UNITS = 4096
MSHARDS = 2
NSHARDS = 4
MS = BATCH // MSHARDS      # 2048 rows of x per core
NS = UNITS // NSHARDS      # 1024 units per core
P = 128
KO = IN_DIM // P           # 32 k-tiles
MP = MS // P               # 16 m-panels per core
NFREE = 512                # matmul moving free dim (one PSUM bank of fp32)
NT = NS // NFREE           # 2 n-tiles per core

_NC_CACHE = {}


def _build(mm_dtype_name="float32r"):
    from concourse import bacc
    import concourse.mybir as mybir
    import concourse.tile as tile
    from concourse.masks import make_identity

    f32 = mybir.dt.float32
    mdt = getattr(mybir.dt, mm_dtype_name)

    nc = bacc.Bacc(None, target_bir_lowering=False, dynamic_dma_scratch_size=8192)

    x_d = nc.dram_tensor("x_s", [MS, IN_DIM], mdt, kind="ExternalInput")
    wmu_d = nc.dram_tensor("wmu_s", [IN_DIM, NS], mdt, kind="ExternalInput")
    wsig_d = nc.dram_tensor("wsig_s", [IN_DIM, NS], f32, kind="ExternalInput")
    bmu_d = nc.dram_tensor("bmu_s", [NS], f32, kind="ExternalInput")
    bsig_d = nc.dram_tensor("bsig_s", [NS], f32, kind="ExternalInput")
    eout_d = nc.dram_tensor("eout_s", [NS], f32, kind="ExternalInput")
    ein_d = nc.dram_tensor("eps_in", [IN_DIM], f32, kind="ExternalInput")
    out_d = nc.dram_tensor("out_s", [MS, NS], f32, kind="ExternalOutput")

    mult = mybir.AluOpType.mult

    WG = 4            # w-group: k-tiles per w tile (wmu DMA granularity 4MB)
    NWG = KO // 8     # 4 w tiles of [P, 8, NS]
    TG = 4            # transposes packed per PSUM bank
    NTG = KO // TG    # 8 transpose groups per panel
    WSC = 2           # wsig staging chunk k-tiles

    with tile.TileContext(nc) as tc:
        with (
            tc.tile_pool(name="const", bufs=1) as const,
            tc.tile_pool(name="wpool", bufs=1) as wpool,
            tc.tile_pool(name="wsig", bufs=2) as wsigp,
            tc.tile_pool(name="xnat", bufs=1) as xnat,
            tc.tile_pool(name="xt", bufs=2) as xtp,
            tc.tile_pool(name="outp", bufs=2) as outp,
            tc.tile_pool(name="ps", bufs=3, space="PSUM") as psp,
            tc.tile_pool(name="pt", bufs=2, space="PSUM") as ptp,
        ):
            # ---- constants ----
            ident_f = const.tile([P, P], f32, tag="identf")
            make_identity(nc, ident_f)
            if mdt != f32:
                ident = const.tile([P, P], mdt, tag="ident")
                nc.vector.tensor_copy(out=ident[:], in_=ident_f[:])
            else:
                ident = ident_f

            eps_in_sb = const.tile([P, KO], f32, tag="epsin")
            with nc.allow_non_contiguous_dma(reason="one-time 16KB strided load"):
                nc.sync.dma_start(
                    eps_in_sb[:],
                    ein_d[:].bitcast(f32).rearrange("(ko ki) -> ki ko", ki=P),
                )

            # bias rows broadcast to all partitions straight from DRAM
            eout_b = const.tile([P, NS], f32, tag="eoutb")
            bsg_b = const.tile([P, NS], f32, tag="bsgb")
            b_b = const.tile([P, NS], f32, tag="bb")
            with nc.allow_non_contiguous_dma(reason="one-time row broadcasts"):
                nc.sync.dma_start(eout_b[:], eout_d[None, :].to_broadcast([P, NS]))
                nc.sync.dma_start(bsg_b[:], bsig_d[None, :].to_broadcast([P, NS]))
                nc.sync.dma_start(b_b[:], bmu_d[None, :].to_broadcast([P, NS]))
            # b = b_mu + b_sigma * eps_out
            nc.vector.tensor_mul(bsg_b[:], bsg_b[:], eout_b[:])
            nc.vector.tensor_add(b_b[:], b_b[:], bsg_b[:])

            # ---- materialize noisy W shard in SBUF: w = w_mu + w_sigma*outer ----
            wmu_r = wmu_d[:].rearrange("(ko ki) n -> ki ko n", ki=P)
            wsig_r = wsig_d[:].rearrange("(ko ki) n -> ki ko n", ki=P)
            w_groups = []
            for g in range(NWG):
                wt = wpool.tile([P, 8, NS], mdt, tag=f"w{g}")
                nc.sync.dma_start(wt[:], wmu_r[:, g * 8 : (g + 1) * 8, :])
                w_groups.append(wt)

            def w_slice(ko, nt):
                return w_groups[ko // 8][
                    :, ko % 8, nt * NFREE : (nt + 1) * NFREE
                ]

            for c in range(KO // WSC):
                ws = wsigp.tile([P, WSC, NS], f32, tag="ws")
                nc.sync.dma_start(ws[:], wsig_r[:, c * WSC : (c + 1) * WSC, :])
                for j in range(WSC):
                    ko = c * WSC + j
                    # ws_j = (eps_out_bcast * eps_in[:,ko]) * ws_j
                    nc.vector.scalar_tensor_tensor(
                        out=ws[:, j, :],
                        in0=eout_b[:],
                        scalar=eps_in_sb[:, ko : ko + 1],
                        in1=ws[:, j, :],
                        op0=mult,
                        op1=mult,
                    )
                    wg = w_groups[ko // 8]
                    nc.vector.tensor_add(
                        wg[:, ko % 8, :], wg[:, ko % 8, :], ws[:, j, :]
                    )

            # ---- panels: transpose x tiles on PE (packed 4/bank), matmuls ----
            def make_transpose_ops(pm):
                xa = xnat.tile([P, IN_DIM // 2], mdt, tag="xa")
                nc.sync.dma_start(xa[:], x_d[pm * P : (pm + 1) * P, 0 : IN_DIM // 2])
                xb = xnat.tile([P, IN_DIM // 2], mdt, tag="xb")
                nc.sync.dma_start(
                    xb[:], x_d[pm * P : (pm + 1) * P, IN_DIM // 2 : IN_DIM]
                )
                xts = [None] * NTG
                ops = []

                def mk(g):
                    def op():
                        pt = ptp.tile([P, TG * P], mdt, tag="pt")
                        for j in range(TG):
                            ko = g * TG + j
                            half = xa if ko < KO // 2 else xb
                            jj = ko % (KO // 2)
                            src = half[:, jj * P : (jj + 1) * P]
                            nc.tensor.matmul(
                                pt[:, j * P : (j + 1) * P],
                                src,
                                ident[:],
                                is_transpose=True,
                                start=(j == 0),
                                stop=(j == TG - 1),
                            )
                        t = xtp.tile([P, TG * P], mdt, tag=f"xt{g}")
                        if g % 2 == 0:
                            nc.vector.tensor_copy(out=t[:], in_=pt[:])
                        else:
                            nc.scalar.copy(out=t[:], in_=pt[:])
                        xts[g] = t

                    return op

                for g in range(NTG):
                    ops.append(mk(g))
                return ops, xts

            def lhsT(xts, ko):
                return xts[ko // TG][:, (ko % TG) * P : (ko % TG + 1) * P]

            prev_xts = None
            for mi in range(MP + 1):
                if mi < MP:
                    t_ops, cur_xts = make_transpose_ops(mi)
                else:
                    t_ops, cur_xts = [], None

                if prev_xts is None:
                    for op in t_ops:
                        op()
                else:
                    pm = mi - 1
                    ti = 0
                    ot = outp.tile([P, NS], f32, tag="ot")
                    for nt in range(NT):
                        ps = psp.tile([P, NFREE], f32, tag="ps")
                        for ko in range(KO):
                            nc.tensor.matmul(
                                ps[:],
                                lhsT(prev_xts, ko),
                                w_slice(ko, nt),
                                start=(ko == 0),
                                stop=(ko == KO - 1),
                            )
                            if ko % 8 == 7 and ti < len(t_ops):
                                t_ops[ti]()
                                ti += 1
                        nc.vector.tensor_add(
                            ot[:, nt * NFREE : (nt + 1) * NFREE],
                            ps[:],
                            b_b[:, nt * NFREE : (nt + 1) * NFREE],
                        )
                    nc.vector.tensor_scalar_max(ot[:], ot[:], 0.0)
                    nc.sync.dma_start(out_d[pm * P : (pm + 1) * P, :], ot[:])
                    while ti < len(t_ops):
                        t_ops[ti]()
                        ti += 1
                prev_xts = cur_xts

    nc.compile()
    return nc


def get_nc(mm_dtype_name="float32r"):
    if mm_dtype_name not in _NC_CACHE:
        _NC_CACHE[mm_dtype_name] = _build(mm_dtype_name)
    return _NC_CACHE[mm_dtype_name]


def shard_inputs(x, w_mu, w_sigma, b_mu, b_sigma, eps_in, eps_out):
    x = np.asarray(x, dtype=np.float32)
    w_mu = np.asarray(w_mu, dtype=np.float32)
    w_sigma = np.asarray(w_sigma, dtype=np.float32)
    b_mu = np.asarray(b_mu, dtype=np.float32)
    b_sigma = np.asarray(b_sigma, dtype=np.float32)
    eps_in = np.asarray(eps_in, dtype=np.float32)
    eps_out = np.asarray(eps_out, dtype=np.float32)

    in_maps = []
    for c in range(MSHARDS * NSHARDS):
        mr, ncol = divmod(c, NSHARDS)
        msl = slice(mr * MS, (mr + 1) * MS)
        nsl = slice(ncol * NS, (ncol + 1) * NS)
        in_maps.append(
            {
                "x_s": np.ascontiguousarray(x[msl, :]),
                "wmu_s": np.ascontiguousarray(w_mu[:, nsl]),
                "wsig_s": np.ascontiguousarray(w_sigma[:, nsl]),
                "bmu_s": np.ascontiguousarray(b_mu[nsl]),
                "bsig_s": np.ascontiguousarray(b_sigma[nsl]),
                "eout_s": np.ascontiguousarray(eps_out[nsl]),
                "eps_in": eps_in,
            }
        )
    return in_maps


def unshard_output(results):
    out = np.empty((BATCH, UNITS), dtype=np.float32)
    for c, rmap in enumerate(results):
        mr, ncol = divmod(c, NSHARDS)
        out[mr * MS : (mr + 1) * MS, ncol * NS : (ncol + 1) * NS] = rmap["out_s"]
    return out


def kernel(x, w_mu, w_sigma, b_mu, b_sigma, eps_in, eps_out):
    from concourse.bass_utils import run_bass_kernel_spmd

    nc = get_nc()
    in_maps = shard_inputs(x, w_mu, w_sigma, b_mu, b_sigma, eps_in, eps_out)
    res = run_bass_kernel_spmd(nc, in_maps, core_ids=list(range(8)))
    return unshard_output(res.results)
